# revision 2
# baseline (speedup 1.0000x reference)
"""Anisotropic upsampling kernel for Trainium2 (8 NeuronCores, batch-sharded).

Computes, for inputs x0 (8,64,64,256), x1 (8,64,128,128), x2 (8,64,256,64):
    out0 = (up_h(x0) + up_w(x1)) / 2   -> (8,64,128,256)
    out1 = (up_h(x1) + up_w(x2)) / 2   -> (8,64,256,128)
where up() is the stride-2, length-5 normalized zero-insert upsampler:
    up(x)[2m]   = (x[m-1]+x[m]+x[m+1])/3   (edges: mean of the 2 valid taps)
    up(x)[2m+1] = (x[m]+x[m+1])/2          (edge m=W-1: x[W-1])

Per core (one batch element):
  - up_h branches run on the TensorEngine as banded-matrix matmuls
    (contraction over H, which sits on SBUF partitions; fp32r at 1 cyc/row).
  - up_w branches run as free-axis stencils: tensor_add on VectorE for the
    2-/3-tap sums, scaled strided writes on ScalarE to interleave even/odd
    output columns.
  - The two branches merge with a single VectorE add reading PSUM directly.
"""

import numpy as np

CC = 8  # channels per chunk


def _up_matrix(H):
    """(2H, H) matrix of the normalized upsampler along one axis."""
    U = np.zeros((2 * H, H), dtype=np.float64)
    for m in range(H):
        taps = [t for t in (m - 1, m, m + 1) if 0 <= t < H]
        for t in taps:
            U[2 * m, t] = 1.0 / len(taps)
        taps = [t for t in (m, m + 1) if 0 <= t < H]
        for t in taps:
            U[2 * m + 1, t] = 1.0 / len(taps)
    return U


_NC_CACHE = {}


def _build():
    import concourse.bass as bass
    import concourse.mybir as mybir
    from concourse import bacc
    from concourse.tile import TileContext

    f32 = mybir.dt.float32

    nc = bacc.Bacc("TRN2", target_bir_lowering=False, debug=False, num_devices=8)

    x0 = nc.dram_tensor("x0", [64, 64, 256], f32, kind="ExternalInput")
    x1 = nc.dram_tensor("x1", [64, 128, 128], f32, kind="ExternalInput")
    x2 = nc.dram_tensor("x2", [64, 256, 64], f32, kind="ExternalInput")
    out0 = nc.dram_tensor("out0", [64, 128, 256], f32, kind="ExternalOutput")
    out1 = nc.dram_tensor("out1", [64, 256, 128], f32, kind="ExternalOutput")

    # lhsT weight constants: lhsT[h, h2] = 0.5 * U[h2, h] (merge /2 folded in)
    U0 = (0.5 * _up_matrix(64).T).astype(np.float32)           # (64, 128)
    U1 = (0.5 * _up_matrix(128).T).astype(np.float32)          # (128, 256)
    u0_d = nc.inline_tensor(U0, "u0_const")
    u1a_d = nc.inline_tensor(np.ascontiguousarray(U1[:, :128]), "u1a_const")
    u1b_d = nc.inline_tensor(np.ascontiguousarray(U1[:, 128:]), "u1b_const")

    with TileContext(nc) as tc:
        with (
            tc.tile_pool(name="wpool", bufs=1) as wpool,
            tc.tile_pool(name="inpool", bufs=3) as inpool,
            tc.tile_pool(name="stpool", bufs=2) as stpool,
            tc.tile_pool(name="opool", bufs=2) as opool,
            tc.tile_pool(name="psum", bufs=2, space="PSUM") as pspool,
        ):
            u0 = wpool.tile([64, 128], f32, tag="u0")
            nc.sync.dma_start(out=u0, in_=u0_d[:, :])
            u1a = wpool.tile([128, 128], f32, tag="u1a")
            nc.sync.dma_start(out=u1a, in_=u1a_d[:, :])
            u1b = wpool.tile([128, 128], f32, tag="u1b")
            nc.sync.dma_start(out=u1b, in_=u1b_d[:, :])
            u1 = (u1a, u1b)

            for ci in range(64 // CC):
                c0 = ci * CC

                # ---- shared loads for this channel chunk ----
                X0 = inpool.tile([64, CC, 256], f32, tag="x0")
                nc.sync.dma_start(
                    out=X0, in_=x0[c0:c0 + CC].rearrange("c h w -> h c w"))
                X1 = inpool.tile([128, CC, 128], f32, tag="x1")
                nc.sync.dma_start(
                    out=X1, in_=x1[c0:c0 + CC].rearrange("c h w -> h c w"))

                # ---- out0 unit: PE up_h(x0) + DVE/ACT up_w(x1) ----
                ps0 = pspool.tile([128, CC * 256], f32, tag="ps")
                X0f = X0.rearrange("h c w -> h (c w)")
                for j in range(4):
                    nc.tensor.matmul(
                        ps0[:, j * 512:(j + 1) * 512],
                        u0,
                        X0f[:, j * 512:(j + 1) * 512],
                        start=True, stop=True,
                    )
                s0 = stpool.tile([128, CC, 127], f32, tag="s0")
                nc.vector.tensor_add(s0, X1[:, :, 0:127], X1[:, :, 1:128])
                t0 = stpool.tile([128, CC, 126], f32, tag="t0")
                nc.vector.tensor_add(t0, s0[:, :, 0:126], X1[:, :, 2:128])
                O0 = opool.tile([128, CC, 256], f32, tag="o0")
                nc.scalar.mul(O0[:, :, 1:254:2], s0, 0.25)
                nc.scalar.mul(O0[:, :, 2:254:2], t0, 1.0 / 6.0)
                nc.scalar.mul(O0[:, :, 0:255:254], s0[:, :, 0:127:126], 0.25)
                nc.scalar.mul(O0[:, :, 255:256], X1[:, :, 127:128], 0.5)
                O0f = O0.rearrange("h c w -> h (c w)")
                nc.vector.tensor_add(O0f, O0f, ps0)
                nc.sync.dma_start(
                    out=out0[c0:c0 + CC].rearrange("c h w -> h c w"), in_=O0)

                # ---- out1 units (two h2 halves): PE up_h(x1) + stencil(x2) ----
                X1f = X1.rearrange("h c w -> h (c w)")
                for half in range(2):
                    ps1 = pspool.tile([128, CC * 128], f32, tag="ps")
                    for j in range(2):
                        nc.tensor.matmul(
                            ps1[:, j * 512:(j + 1) * 512],
                            u1[half],
                            X1f[:, j * 512:(j + 1) * 512],
                            start=True, stop=True,
                        )
                    X2 = inpool.tile([128, CC, 64], f32, tag="x2")
                    nc.sync.dma_start(
                        out=X2,
                        in_=x2[c0:c0 + CC, 128 * half:128 * (half + 1)]
                        .rearrange("c h w -> h c w"))
                    s1 = stpool.tile([128, CC, 63], f32, tag="s1")
                    nc.vector.tensor_add(s1, X2[:, :, 0:63], X2[:, :, 1:64])
                    t1 = stpool.tile([128, CC, 62], f32, tag="t1")
                    nc.vector.tensor_add(t1, s1[:, :, 0:62], X2[:, :, 2:64])
                    O1 = opool.tile([128, CC, 128], f32, tag="o1")
                    nc.scalar.mul(O1[:, :, 1:126:2], s1, 0.25)
                    nc.scalar.mul(O1[:, :, 2:126:2], t1, 1.0 / 6.0)
                    nc.scalar.mul(O1[:, :, 0:127:126], s1[:, :, 0:63:62], 0.25)
                    nc.scalar.mul(O1[:, :, 127:128], X2[:, :, 63:64], 0.5)
                    O1f = O1.rearrange("h c w -> h (c w)")
                    nc.vector.tensor_add(O1f, O1f, ps1)
                    nc.sync.dma_start(
                        out=out1[c0:c0 + CC, 128 * half:128 * (half + 1)]
                        .rearrange("c h w -> h c w"),
                        in_=O1)

    nc.compile()
    return nc


def _get_nc():
    if "nc" not in _NC_CACHE:
        _NC_CACHE["nc"] = _build()
    return _NC_CACHE["nc"]


def kernel(x0, x1, x2):
    from concourse.bass_utils import run_bass_kernel_spmd

    nc = _get_nc()
    in_maps = [
        {
            "x0": np.ascontiguousarray(x0[b]),
            "x1": np.ascontiguousarray(x1[b]),
            "x2": np.ascontiguousarray(x2[b]),
        }
        for b in range(8)
    ]
    res = run_bass_kernel_spmd(nc, in_maps, core_ids=list(range(8)))
    o0 = np.stack([res.results[b]["out0"] for b in range(8)])
    o1 = np.stack([res.results[b]["out1"] for b in range(8)])
    return o0, o1
